# revision 1
# baseline (speedup 1.0000x reference)
"""Trainium2 Bass kernel for SlidingWindowAttention3d (3x3x3 window, D8 H56 W56, 8 heads).

Sharding: 8 cores = batch(4) x z-halves(2). Each core owns 12544 tokens
(4 z-planes of 56x56) and receives a z-halo in its input slab; cores are
fully independent (no collectives). One SPMD program for all cores; the
z-boundary mask is data-driven (per-core exp-bias table: rpb or -200).

On-chip layout: channels (C=128 = 8 heads x 16 dims) on partitions, tokens on
the free axis.  A 3D window offset (dz,dy,dx) is a linear shift
dz*3136 + dy*56 + dx along the token axis; y/x wrap-arounds read a wrong-but-
finite neighbor and are zeroed in the attention weights afterwards (exactly
what the reference's -inf mask achieves).

Per 448-token tile (8 y-rows; one z-plane covers 7 tiles):
  q-pipeline: PE proj -> ACT square -> PE block-ones sumsq -> ACT sqrt ->
              DVE recip -> fused scale/embed -> seq-len scale
  per offset j (27): DVE Hadamard q*k_shift (fp16) -> PE block-ones reduce
              (head sums, d-duplicated) -> ACT exp(S + biastab[t,j]) -> bf16 A
              -> boundary memsets -> DVE A*v_shift -> PE identity accumulate
              (AV sum) + PE block-ones/16 accumulate (softmax denominator)
  epilogue:   DVE reciprocal of denominator, weight the AV sum, PE out-proj.
"""

from contextlib import ExitStack

import numpy as np
import ml_dtypes

import concourse.bass as bass
from concourse import bacc
import concourse.mybir as mybir
import concourse.tile as tile
from concourse.bass_utils import run_bass_kernel_spmd

F32 = mybir.dt.float32
F16 = mybir.dt.float16
BF16 = mybir.dt.bfloat16

D, H, W = 8, 56, 56
NH, HD, C = 8, 16, 128
N = D * H * W            # 25088
B = 4
NCORES = 8
PLANE = H * W            # 3136
NOWN = 4 * PLANE         # 12544 tokens per core
PAD = 3584               # z-halo pad (>= 3137+56+1 = 3194), multiple of 448
NEXT = NOWN + 2 * PAD    # 19712
T = 448                  # tile: 8 y-rows
NT = NOWN // T           # 28 tiles per core
import os as _os
_NT_LIMIT = int(_os.environ.get("K_NT_LIMIT", NT))
NCH = NEXT // T          # 44 prologue chunks
TILES_PER_PLANE = 7
NJ = 27


def _win(tens_ap, off, dims):
    return bass.AP(tens_ap.tensor, off,
                   [list(tens_ap.ap[0])] + [list(d) for d in dims])


def _patch_act_tables():
    """Force all ACT funcs onto one table set (natural_log_exp_and_others holds
    Exp/Ln/Square/Identity/Copy) so no per-tile table reloads are emitted."""
    import concourse.hw_specs as hw_specs
    if getattr(hw_specs, "_ant_act_tables_patched", False):
        return
    orig = hw_specs.get_activation_tables

    def patched(module_arch):
        tabs = dict(orig(module_arch))
        keep = "natural_log_exp_and_others"
        if keep in tabs:
            tabs = {k: (v if k == keep else set()) for k, v in tabs.items()}
        return tabs

    hw_specs.get_activation_tables = patched
    bacc.get_activation_tables = patched
    hw_specs._ant_act_tables_patched = True


def _build_nc() -> bass.Bass:
    _patch_act_tables()
    nc = bacc.Bacc("TRN2")

    x_ext = nc.dram_tensor("x_ext", [C, NEXT], F16, kind="ExternalInput")
    w_q = nc.dram_tensor("w_q", [C, C], F16, kind="ExternalInput")
    w_k = nc.dram_tensor("w_k", [C, C], F16, kind="ExternalInput")
    w_v = nc.dram_tensor("w_v", [C, C], F16, kind="ExternalInput")
    w_p = nc.dram_tensor("w_p", [C, C], F16, kind="ExternalInput")
    odup = nc.dram_tensor("odup", [C, C], F16, kind="ExternalInput")
    odup16 = nc.dram_tensor("odup16", [C, C], F16, kind="ExternalInput")
    ident = nc.dram_tensor("ident", [C, C], BF16, kind="ExternalInput")
    kb = nc.dram_tensor("kb", [C, 1], F32, kind="ExternalInput")
    vb = nc.dram_tensor("vb", [C, 1], F32, kind="ExternalInput")
    qb = nc.dram_tensor("qb", [C, 1], F32, kind="ExternalInput")
    pb = nc.dram_tensor("pb", [C, 1], F32, kind="ExternalInput")
    qsc = nc.dram_tensor("qsc", [C, 1], F32, kind="ExternalInput")
    qbi = nc.dram_tensor("qbi", [C, 1], F32, kind="ExternalInput")
    # per-core exp bias: rpb[h,j] (+ -200 where the dz plane is out of volume)
    rpbt = nc.dram_tensor("rpbt", [C, NT * NJ], F32, kind="ExternalInput")
    ssb_in = nc.dram_tensor("ssb", [C, NOWN], F16, kind="ExternalInput")
    out = nc.dram_tensor("out", [C, NOWN], F32, kind="ExternalOutput")

    with tile.TileContext(nc) as tc, ExitStack() as ctx:
        singles = ctx.enter_context(tc.tile_pool(name="singles", bufs=1))

        k_ext = singles.tile([C, NEXT], F16, tag="k_ext")
        v_ext = singles.tile([C, NEXT], BF16, tag="v_ext")

        sb_wq = singles.tile([C, C], F16, tag="wq")
        sb_wk = singles.tile([C, C], F16, tag="wk")
        sb_wv = singles.tile([C, C], F16, tag="wv")
        sb_wp = singles.tile([C, C], F16, tag="wp")
        sb_od = singles.tile([C, C], F16, tag="od")
        sb_od16 = singles.tile([C, C], F16, tag="od16")
        sb_id = singles.tile([C, C], BF16, tag="id")
        sb_kb = singles.tile([C, 1], F32, tag="kb")
        sb_vb = singles.tile([C, 1], F32, tag="vb")
        sb_qb = singles.tile([C, 1], F32, tag="qb")
        sb_pb = singles.tile([C, 1], F32, tag="pb")
        sb_qsc = singles.tile([C, 1], F32, tag="qsc")
        sb_qbi = singles.tile([C, 1], F32, tag="qbi")
        sb_rpbt = singles.tile([C, NT * NJ], F32, tag="rpbt")
        sb_ssb = singles.tile([C, NOWN], F16, tag="ssb")
        sb_eps = singles.tile([C, 1], F32, tag="eps")

        nc.sync.dma_start(out=sb_wq, in_=w_q[:, :])
        nc.sync.dma_start(out=sb_wk, in_=w_k[:, :])
        nc.sync.dma_start(out=sb_wv, in_=w_v[:, :])
        nc.sync.dma_start(out=sb_wp, in_=w_p[:, :])
        nc.sync.dma_start(out=sb_od, in_=odup[:, :])
        nc.sync.dma_start(out=sb_od16, in_=odup16[:, :])
        nc.sync.dma_start(out=sb_id, in_=ident[:, :])
        nc.sync.dma_start(out=sb_kb, in_=kb[:, :])
        nc.sync.dma_start(out=sb_vb, in_=vb[:, :])
        nc.sync.dma_start(out=sb_qb, in_=qb[:, :])
        nc.sync.dma_start(out=sb_pb, in_=pb[:, :])
        nc.sync.dma_start(out=sb_qsc, in_=qsc[:, :])
        nc.sync.dma_start(out=sb_qbi, in_=qbi[:, :])
        nc.sync.dma_start(out=sb_rpbt, in_=rpbt[:, :])
        nc.sync.dma_start(out=sb_ssb, in_=ssb_in[:, :])
        nc.vector.memset(sb_eps, 1e-24)

        # ---------- k / v production (interleaved with main loop) ----------
        def kv_chunk(ch, px, pst):
                c0 = ch * T
                xc = px.tile([C, T], F16, tag="xc")
                nc.sync.dma_start(out=xc, in_=x_ext[:, c0 : c0 + T])

                kp = psum.tile([C, T], F32, tag="smallmm", bufs=4)
                nc.tensor.matmul(kp, lhsT=sb_wk, rhs=xc, start=True, stop=True)
                vp = psum.tile([C, T], F32, tag="smallmm", bufs=4)
                nc.tensor.matmul(vp, lhsT=sb_wv, rhs=xc, start=True, stop=True)

                nc.vector.tensor_scalar(
                    out=v_ext[:, c0 : c0 + T], in0=vp, scalar1=sb_vb, scalar2=None,
                    op0=mybir.AluOpType.add,
                )

                kpb = pst.tile([C, T], F16, tag="kpb", bufs=2)
                nc.vector.tensor_scalar(
                    out=kpb, in0=kp, scalar1=sb_kb, scalar2=None,
                    op0=mybir.AluOpType.add,
                )
                sqk = pst.tile([C, T], F16, tag="sqk")
                nc.vector.tensor_mul(sqk, kpb, kpb)
                ssq = psum.tile([C, T], F32, tag="smallmm", bufs=4)
                nc.tensor.matmul(ssq, lhsT=sb_od, rhs=sqk, start=True, stop=True)
                nrm = pst.tile([C, T], F32, tag="pnorm", bufs=4)
                nc.scalar.activation(
                    out=nrm, in_=ssq,
                    func=mybir.ActivationFunctionType.Ln, bias=sb_eps, scale=1.0,
                )
                rs = pst.tile([C, T], F16, tag="pnorm2", bufs=2)
                nc.scalar.activation(
                    out=rs, in_=nrm,
                    func=mybir.ActivationFunctionType.Exp, bias=0.0, scale=-0.5,
                )
                nc.vector.tensor_mul(k_ext[:, c0 : c0 + T], kpb, rs)

        # ---------- main loop over 28 tiles ----------
        with (
            tc.tile_pool(name="px", bufs=3) as px,
            tc.tile_pool(name="pst", bufs=3) as pst,
            tc.tile_pool(name="mx", bufs=3) as mx,
            tc.tile_pool(name="mq", bufs=2) as mq,
            tc.tile_pool(name="ma", bufs=2) as ma,
            tc.tile_pool(name="mp", bufs=4) as mp,
            tc.tile_pool(name="mo", bufs=2) as mo,
            tc.tile_pool(name="psum", bufs=1, space="PSUM") as psum,
        ):
            HEAD_CHUNKS = 18
            for ch in range(min(HEAD_CHUNKS, NCH)):
                kv_chunk(ch, px, pst)

            qf_ring = {}

            def qpipe(t):
                n0 = t * T
                e0 = PAD + n0
                xq = mx.tile([C, T], F16, tag="xq")
                nc.sync.dma_start(out=xq, in_=x_ext[:, e0 : e0 + T])
                qp = psum.tile([C, T], F32, tag="smallmm", bufs=4)
                nc.tensor.matmul(qp, lhsT=sb_wq, rhs=xq, start=True, stop=True)
                sqq = mq.tile([C, T], F16, tag="sqq")
                nc.scalar.activation(
                    out=sqq, in_=qp,
                    func=mybir.ActivationFunctionType.Square, bias=sb_qb, scale=1.0,
                )
                ssqq = psum.tile([C, T], F32, tag="smallmm", bufs=4)
                nc.tensor.matmul(ssqq, lhsT=sb_od, rhs=sqq, start=True, stop=True)
                nrmq = mq.tile([C, T], F32, tag="qtmp", bufs=3)
                nc.scalar.activation(
                    out=nrmq, in_=ssqq,
                    func=mybir.ActivationFunctionType.Ln, bias=sb_eps, scale=1.0,
                )
                rsq = mq.tile([C, T], F32, tag="qtmp", bufs=3)
                nc.scalar.activation(
                    out=rsq, in_=nrmq,
                    func=mybir.ActivationFunctionType.Exp, bias=0.0, scale=-0.5,
                )
                q1 = mq.tile([C, T], F32, tag="qtmp", bufs=3)
                nc.vector.scalar_tensor_tensor(
                    out=q1, in0=qp, scalar=sb_qb, in1=rsq,
                    op0=mybir.AluOpType.add, op1=mybir.AluOpType.mult,
                )
                q2 = mq.tile([C, T], F16, tag="sqq", bufs=2)
                nc.vector.tensor_scalar(
                    out=q2, in0=q1, scalar1=sb_qsc, scalar2=sb_qbi,
                    op0=mybir.AluOpType.mult, op1=mybir.AluOpType.add,
                )
                qf = mq.tile([C, T], F16, tag="qf", bufs=4)
                nc.vector.tensor_mul(qf, q2, sb_ssb[:, n0 : n0 + T])
                qf_ring[t] = qf

            def phases(t):
                n0 = t * T
                e0 = PAD + n0
                r = t % TILES_PER_PLANE
                qf = qf_ring.pop(t)

                a_all = ma.tile([C, NJ, T], BF16, tag="a_all")
                avacc = psum.tile([C, T], F32, tag="avacc", bufs=1)
                dend = psum.tile([C, T], F32, tag="dend", bufs=1)

                edge = t < TILES_PER_PLANE  # plane 0 = z edge (mirrored half1)
                js = [
                    (dzi * 9 + dyi * 3 + dxi, dzi, dyi, dxi)
                    for dzi in range(3) for dyi in range(3) for dxi in range(3)
                    if not (edge and dzi == 0)
                ]
                first_j, last_j = js[0][0], js[-1][0]

                # phase 1: batched QK hadamards (3 dx per DVE op) + exp
                qf_bc = qf[:, :].unsqueeze(1).broadcast_to([C, 3, T])
                for dzi in range(3):
                    if edge and dzi == 0:
                        continue
                    for dyi in range(3):
                        off = e0 + (dzi - 1) * PLANE + (dyi - 1) * W - 1
                        pj3 = mp.tile([C, 3, T], F16, tag="pj3", bufs=3)
                        nc.vector.tensor_mul(
                            pj3, qf_bc, _win(k_ext[:, :], off, [[1, 3], [1, T]]))
                        for dxi in range(3):
                            j = dzi * 9 + dyi * 3 + dxi
                            sd = psum.tile([C, T], F32, tag="sd", bufs=2)
                            nc.tensor.matmul(sd, lhsT=sb_od, rhs=pj3[:, dxi, :],
                                             start=True, stop=True)
                            nc.scalar.activation(
                                out=a_all[:, j, :], in_=sd,
                                func=mybir.ActivationFunctionType.Exp,
                                bias=sb_rpbt[:, t * NJ + j : t * NJ + j + 1],
                                scale=1.0,
                            )
                            if dyi == 0 and r == 0:
                                nc.gpsimd.memset(a_all[:, j, 0:W], 0.0)
                            if dyi == 2 and r == TILES_PER_PLANE - 1:
                                nc.gpsimd.memset(a_all[:, j, T - W : T], 0.0)
                            av = a_all[:, j, :].rearrange("p (rr x) -> p rr x", x=W)
                            if dxi == 0:
                                nc.gpsimd.memset(av[:, :, 0:1], 0.0)
                            if dxi == 2:
                                nc.gpsimd.memset(av[:, :, W - 1 : W], 0.0)

                # phase 2: softmax denominator (PE accumulate over j)
                for j, dzi, dyi, dxi in js:
                    nc.tensor.matmul(
                        dend, lhsT=sb_od16, rhs=a_all[:, j, :],
                        start=(j == first_j), stop=(j == last_j),
                    )

                # phases 3+4: batched AV hadamards (3 dx per DVE op)
                for dzi in range(3):
                    if edge and dzi == 0:
                        continue
                    for dyi in range(3):
                        off = e0 + (dzi - 1) * PLANE + (dyi - 1) * W - 1
                        j0 = dzi * 9 + dyi * 3
                        avp3 = mp.tile([C, 3, T], BF16, tag="avp3", bufs=3)
                        nc.vector.tensor_mul(
                            avp3, a_all[:, j0:j0 + 3, :],
                            _win(v_ext[:, :], off, [[1, 3], [1, T]]))
                        for dxi in range(3):
                            j = j0 + dxi
                            nc.tensor.matmul(
                                avacc, lhsT=sb_id, rhs=avp3[:, dxi, :],
                                start=(j == first_j), stop=(j == last_j),
                            )

                # epilogue
                rd = mo.tile([C, T], F32, tag="epi", bufs=4)
                scr2 = mo.tile([C, T], F32, tag="epi", bufs=4)
                nc.vector.reciprocal_approx_accurate(out=rd, in_=dend, scratch=scr2)
                outt = mo.tile([C, T], F16, tag="outt")
                nc.vector.tensor_mul(outt, avacc, rd)
                projp = psum.tile([C, T], F32, tag="smallmm", bufs=4)
                nc.tensor.matmul(projp, lhsT=sb_wp, rhs=outt, start=True, stop=True)
                osb = mo.tile([C, T], F32, tag="epi", bufs=4)
                nc.scalar.activation(
                    out=osb, in_=projp,
                    func=mybir.ActivationFunctionType.Identity, bias=sb_pb, scale=1.0,
                )
                nc.sync.dma_start(out=out[:, n0 : n0 + T], in_=osb)

            for t in range(_NT_LIMIT):
                ch = HEAD_CHUNKS + t
                if ch < NCH:
                    kv_chunk(ch, px, pst)
                qpipe(t)
                if t >= 2:
                    phases(t - 2)
            for t in range(max(_NT_LIMIT - 2, 0), _NT_LIMIT):
                phases(t)

    nc.finalize()
    return nc


_NC_CACHE: list = []


def _get_nc() -> bass.Bass:
    if not _NC_CACHE:
        _NC_CACHE.append(_build_nc())
    return _NC_CACHE[0]


def _softplus(x):
    return np.log1p(np.exp(x))


def _host_prep(inputs):
    x = np.asarray(inputs["x"], np.float32)          # [B, N, C]
    q_w = np.asarray(inputs["q_w"], np.float32)      # [C, C]
    q_b = np.asarray(inputs["q_b"], np.float32)
    kv_w = np.asarray(inputs["kv_w"], np.float32)    # [2C, C]
    kv_b = np.asarray(inputs["kv_b"], np.float32)
    proj_w = np.asarray(inputs["proj_w"], np.float32)
    proj_b = np.asarray(inputs["proj_b"], np.float32)
    temp = np.asarray(inputs["temperature"], np.float32).reshape(NH)
    qe = np.asarray(inputs["query_embedding"], np.float32).reshape(NH, HD)
    rpb = np.asarray(inputs["rel_pos_bias"], np.float32)  # [NH, 27]

    sp = _softplus(temp)
    qsc = np.repeat(sp, HD).reshape(C, 1).astype(np.float32)
    qbi = (qe * sp[:, None]).reshape(C, 1).astype(np.float32)
    rpb_dup = np.repeat(rpb, HD, axis=0).astype(np.float32)  # [C, 27]

    def span(i, L):
        return 3 - (i == 0) - (i == L - 1)
    z = np.arange(D)[:, None, None]
    y = np.arange(H)[None, :, None]
    xx = np.arange(W)[None, None, :]
    cnt = span(z, D) * span(y, H) * span(xx, W)
    ss_full = np.log(cnt.astype(np.float32)).reshape(N)

    blk = np.zeros((C, C), np.float32)
    for h in range(NH):
        blk[h * HD : (h + 1) * HD, h * HD : (h + 1) * HD] = 1.0

    common = {
        "w_q": q_w.T.astype(np.float16),
        "w_k": kv_w[:C].T.astype(np.float16),
        "w_v": kv_w[C:].T.astype(np.float16),
        "w_p": proj_w.T.astype(np.float16),
        "odup": blk.astype(np.float16),
        "odup16": (blk / 16.0).astype(np.float16),
        "ident": np.eye(C, dtype=np.float32).astype(ml_dtypes.bfloat16),
        "kb": kv_b[:C].reshape(C, 1).astype(np.float32),
        "vb": kv_b[C:].reshape(C, 1).astype(np.float32),
        "qb": q_b.reshape(C, 1).astype(np.float32),
        "pb": proj_b.reshape(C, 1).astype(np.float32),
        "qsc": qsc, "qbi": qbi,
    }

    in_maps = []
    for core in range(NCORES):
        b, half = core // 2, core % 2
        # half 1 processes its z-range mirrored (token order reversed) so the
        # z-edge is always at plane index 0; offset j maps to 26-j.
        if half == 0:
            xb = x[b]
            ss_c = ss_full[:NOWN]
            rpb_c = rpb_dup
        else:
            xb = x[b, ::-1, :]
            ss_c = ss_full[::-1][:NOWN]
            rpb_c = rpb_dup[:, ::-1]

        # rpbt indexed [tile, j]; same bias for every tile now, but keep layout
        rpbt = np.tile(rpb_c[:, None, :], (1, NT, 1))
        rpbt = np.ascontiguousarray(rpbt.reshape(C, NT * NJ), dtype=np.float32)

        xt = np.zeros((C, NEXT), np.float16)
        lo, hi = -PAD, NOWN + PAD
        src_lo, src_hi = max(lo, 0), min(hi, N)
        xt[:, src_lo - lo : src_hi - lo] = xb[src_lo:src_hi, :].T.astype(np.float16)

        m = dict(common)
        m["x_ext"] = xt
        m["rpbt"] = rpbt
        m["ssb"] = np.ascontiguousarray(
            np.broadcast_to(ss_c.astype(np.float16)[None, :], (C, NOWN))
        )
        in_maps.append(m)
    return in_maps


def kernel(**inputs) -> np.ndarray:
    in_maps = _host_prep(inputs)
    nc = _get_nc()
    res = run_bass_kernel_spmd(nc, in_maps, core_ids=list(range(NCORES)))
    out_full = np.zeros((B, N, C), np.float32)
    for core in range(NCORES):
        b, half = core // 2, core % 2
        o = res.results[core]["out"].T  # [NOWN, C] in (possibly mirrored) order
        if half == 0:
            out_full[b, :NOWN, :] = o
        else:
            out_full[b, NOWN:, :] = o[::-1, :]
    return out_full

